# revision 1
# baseline (speedup 1.0000x reference)
"""DGCNN (3x DynamicEdgeConv + point MLP) Trainium2 kernel.

Self-contained: builds a Bass/Tile program that processes 2 point clouds per
NeuronCore and runs it SPMD on 8 cores (data-parallel over the batch of 16).

Algorithm per cloud, per edge-conv layer (feature-major layout XT [C, N]):
  S = 2*X@X.T - |x_j|^2          (row-shifted -distance; same per-row top-k)
  top-30 neighbor selection on DVE:
     pack local column index (8 bits, complemented) into S's mantissa LSBs,
     screen top-16 per 256-chunk with Max8/MatchReplace, 4 merge rounds with
     MaxIndex position recovery (chunk id from candidate position).
  gather neighbor/self features via GPSIMD indirect_copy (per-core wrapped
  index lists), edge MLP via TensorE (h1 = relu(W@[xj;xi]+b0), h2 = w1@h1),
  max-aggregate over the 30 edges via strided tensor_reduce.
Then the pointwise MLP 192->1024->256->128->3 on TensorE.
"""
import sys
import numpy as np

for _p in ("/opt/trn_rl_repo", "/root/.axon_site/_ro/trn_rl_repo"):
    if _p not in sys.path:
        sys.path.append(_p)

B, N, KNN = 16, 2048, 30
NCORES = 8
CPC = B // NCORES          # clouds per core
P = 128                    # partitions
NT = N // P                # row tiles per cloud (16)
CHUNK = 256                # selection screen chunk
NCH = N // CHUNK           # 8 chunks
DEPTH = 16                 # per-chunk screen depth
NB = 8                     # packed local-index bits
KSEL = 32                  # extracted per row (use first 30)
SUB = 16                   # points per edge sub-chunk
ESUB = SUB * KNN           # 480 edge slots per sub-chunk
NSUB = P // SUB            # 8 sub-chunks per tile
NEG = -3.0e38

_PROGRAM_CACHE = {}


def _build_program():
    import concourse.bass as bass
    import concourse.bacc as bacc
    import concourse.tile as tile
    from concourse import mybir
    from contextlib import ExitStack

    f32 = mybir.dt.float32
    u32 = mybir.dt.uint32
    u16 = mybir.dt.uint16
    Alu = mybir.AluOpType
    Act = mybir.ActivationFunctionType

    nc = bacc.Bacc()

    # ---------------- DRAM parameters ----------------
    def din(name, shape, dtype=f32):
        return nc.declare_dram_parameter(name, list(shape), dtype, isOutput=False)

    posT_d = din("posT", [CPC, 3, N])
    iota_d = din("iota_lc", [P, N], u32)
    iwrap_d = din("iwrap", [NT, 16, 8 * KNN], u16)
    conv_w = []
    for l, c in ((1, 3), (2, 64), (3, 64)):
        conv_w.append(dict(
            C=c,
            w0a=din(f"c{l}_w0a", [c, 64]),
            w0b=din(f"c{l}_w0b", [c, 64]),
            w1=din(f"c{l}_w1", [64, 64]),
            b0=din(f"c{l}_b0", [64, 1]),
            b1=din(f"c{l}_b1", [64, 1]),
        ))
    mw0k0_d = din("mlp_w0k0", [128, 1024])
    mw0k1_d = din("mlp_w0k1", [64, 1024])
    mb0_d = din("mlp_b0", [128, 8])
    mw1_d = din("mlp_w1r", [128, 8, 256])
    mb1_d = din("mlp_b1", [128, 2])
    mw2_d = din("mlp_w2r", [128, 2, 128])
    mb2_d = din("mlp_b2", [128, 1])
    finw_d = din("fin_w", [128, 3])
    finb_d = din("fin_brep", [128, 3])
    out_d = nc.declare_dram_parameter("out", [CPC, N, 3], f32, isOutput=True)

    with tile.TileContext(nc) as tc, ExitStack() as CTX:
        # ------------- persistent pools -------------
        persist = CTX.enter_context(tc.tile_pool(name="persist", bufs=1))

        iota_lc = persist.tile([P, N], u32)
        nc.sync.dma_start(iota_lc, iota_d[:])

        # per-layer H1pre weights (built on device), w1, biases
        wh1 = []
        for l in range(3):
            cw = conv_w[l]
            C = cw["C"]
            w0a = persist.tile([C, 64], f32, name=f"w0a_{l}")
            nc.sync.dma_start(w0a, cw["w0a"][:])
            w0b = persist.tile([C, 64], f32, name=f"w0b_{l}")
            nc.sync.dma_start(w0b, cw["w0b"][:])
            w0diff = persist.tile([C, 64], f32, name=f"w0diff_{l}")
            nc.vector.tensor_sub(w0diff, w0a, w0b)
            kdim = 32 if l == 0 else 128
            wh = persist.tile([kdim, 64], f32, name=f"wh1_{l}")
            nc.vector.memset(wh, 0.0)
            if l == 0:
                nc.sync.dma_start(wh[0:3, :], w0b)
                nc.sync.dma_start(wh[16:19, :], w0diff)
            else:
                for g in range(4):
                    nc.sync.dma_start(wh[32 * g:32 * g + 16, :], w0b[16 * g:16 * g + 16, :])
                    nc.sync.dma_start(wh[32 * g + 16:32 * g + 32, :], w0diff[16 * g:16 * g + 16, :])
            w1t = persist.tile([64, 64], f32, name=f"w1_{l}")
            nc.sync.dma_start(w1t, cw["w1"][:])
            b0t = persist.tile([64, 1], f32, name=f"b0_{l}")
            nc.sync.dma_start(b0t, cw["b0"][:])
            b1t = persist.tile([64, 1], f32, name=f"b1_{l}")
            nc.sync.dma_start(b1t, cw["b1"][:])
            wh1.append(dict(C=C, wh=wh, kdim=kdim, w1=w1t, b0=b0t, b1=b1t))

        # MLP weights
        mw0k0 = persist.tile([128, 1024], f32)
        nc.sync.dma_start(mw0k0, mw0k0_d[:])
        mw0k1 = persist.tile([64, 1024], f32)
        nc.sync.dma_start(mw0k1, mw0k1_d[:])
        mb0 = persist.tile([128, 8], f32)
        nc.sync.dma_start(mb0, mb0_d[:])
        mw1 = persist.tile([128, 8, 256], f32)
        nc.sync.dma_start(mw1, mw1_d[:])
        mb1 = persist.tile([128, 2], f32)
        nc.sync.dma_start(mb1, mb1_d[:])
        mw2 = persist.tile([128, 2, 128], f32)
        nc.sync.dma_start(mw2, mw2_d[:])
        mb2 = persist.tile([128, 1], f32)
        nc.sync.dma_start(mb2, mb2_d[:])
        finw = persist.tile([128, 3], f32)
        nc.sync.dma_start(finw, finw_d[:])
        finb = persist.tile([128, 3], f32)
        nc.sync.dma_start(finb, finb_d[:])

        ones_col = persist.tile([64, 1], f32)
        nc.vector.memset(ones_col, 1.0)

        # ---------------- per-cloud processing ----------------
        for cloud in range(CPC):
            with ExitStack() as cctx:
                cloudp = cctx.enter_context(tc.tile_pool(name=f"cloud{cloud}", bufs=1))

                # per-tile gather index tensors (i-rows persist across layers)
                idx_tiles = []
                for t in range(NT):
                    it = cloudp.tile([P, 8 * KNN], u16, name=f"idxt{t}", tag="idxt", bufs=NT)
                    for rep in range(4):
                        nc.sync.dma_start(it[32 * rep + 16:32 * rep + 32, :], iwrap_d[t])
                    idx_tiles.append(it)

                # feature tensors (aug: +1 ones row used as S-matmul lhsT rows)
                xt1 = cloudp.tile([33, N], f32)     # layer1 input (pos), row32 = ones
                nc.vector.memset(xt1, 0.0)
                nc.sync.dma_start(xt1[0:3, :], posT_d[cloud])
                nc.vector.memset(xt1[32:33, :], 1.0)
                xt2 = cloudp.tile([65, N], f32)     # x1 + ones row
                nc.vector.memset(xt2[64:65, :], 1.0)
                xt3 = cloudp.tile([65, N], f32)     # x2 + ones row
                nc.vector.memset(xt3[64:65, :], 1.0)
                x12 = cloudp.tile([128, N], f32)    # [x1; x2] for MLP
                x3 = cloudp.tile([64, N], f32)      # x3 for MLP
                xtaugs = [xt1, xt2, xt3]

                for l in range(3):
                    with ExitStack() as lctx:
                        cw = wh1[l]
                        C = cw["C"]
                        augrow = 32 if l == 0 else 64   # partition of the ones/x2 row
                        caug = augrow + 1
                        xtaug = xtaugs[l]
                        xt_next = None if l == 2 else xtaugs[l + 1]

                        lp = lctx.enter_context(tc.tile_pool(name=f"lay{cloud}_{l}", bufs=1))
                        work = lctx.enter_context(tc.tile_pool(name=f"lw{cloud}_{l}", bufs=2))
                        psel = lctx.enter_context(tc.tile_pool(name=f"ps{cloud}_{l}", bufs=2))
                        spsum_p = lctx.enter_context(tc.tile_pool(name=f"sp{cloud}_{l}", bufs=1, space="PSUM"))
                        edge_ps = lctx.enter_context(tc.tile_pool(name=f"ep{cloud}_{l}", bufs=2, space="PSUM"))

                        # ---- R = [2*XT ; -x2] ----
                        R = lp.tile([caug, N], f32)
                        if l == 0:
                            nc.vector.memset(R, 0.0)
                        nc.scalar.activation(R[0:C, :], xtaug[0:C, :], Act.Copy, scale=2.0)
                        sq = lp.tile([C, N], f32)
                        nc.scalar.activation(sq, xtaug[0:C, :], Act.Square)
                        for nchk in range(4):
                            x2ps = spsum_p.tile([1, 512], f32, name=f"x2ps{cloud}_{l}_{nchk}",
                                                tag="spsum")
                            nc.tensor.matmul(x2ps,
                                             lhsT=ones_col[0:C, :],
                                             rhs=sq[:, nchk * 512:(nchk + 1) * 512],
                                             start=True, stop=True)
                            nc.scalar.activation(R[augrow:caug, nchk * 512:(nchk + 1) * 512],
                                                 x2ps, Act.Copy, scale=-1.0)

                        # ---- gather data D [128, N] (interleaved j/i copies) ----
                        D = lp.tile([P, N], f32)
                        if l == 0:
                            nc.vector.memset(D, 0.0)
                            nc.sync.dma_start(D[0:3, :], xtaug[0:3, :])
                            nc.sync.dma_start(D[16:19, :], xtaug[0:3, :])
                        else:
                            for g in range(4):
                                nc.sync.dma_start(D[32 * g:32 * g + 16, :], xtaug[16 * g:16 * g + 16, :])
                                nc.sync.dma_start(D[32 * g + 16:32 * g + 32, :], xtaug[16 * g:16 * g + 16, :])

                        # ---- 3-stage software pipeline over the 16 row tiles:
                        #   A(t): S matmul + PSUM->SBUF copy
                        #   B(t): selection + wrapped-idx DMAs
                        #   C(t): gather + edge MLP + aggregation
                        # Skewed emission keeps every engine's in-order stream
                        # supplied with ready work (2-tile lookahead).
                        scp_tiles = {}

                        def stage_a(t):
                            spsum = spsum_p.tile([P, N], f32, name=f"spsum{cloud}_{l}_{t}", tag="spsum")
                            lhsT = xtaug[:, t * P:(t + 1) * P]
                            for nchk in range(4):
                                nc.tensor.matmul(spsum[:, nchk * 512:(nchk + 1) * 512],
                                                 lhsT=lhsT,
                                                 rhs=R[:, nchk * 512:(nchk + 1) * 512],
                                                 start=True, stop=True)
                            scp = work.tile([P, N], f32, tag="spk", name=f"scp{t}", bufs=3)
                            nc.scalar.activation(scp, spsum, Act.Copy)
                            scp_tiles[t] = scp

                        def stage_b(t):
                            # selection: exact values; indices via full-row max_index;
                            # first-match semantics == top_k's smallest-index tie-break
                            scp = scp_tiles[t]
                            cand = psel.tile([P, NCH * DEPTH], f32, tag="cand")
                            scr = psel.tile([P, CHUNK], f32, tag="scr")
                            for c in range(NCH):
                                chunk = scp[:, c * CHUNK:(c + 1) * CHUNK]
                                nc.vector.max(out=cand[:, c * DEPTH:c * DEPTH + 8], in_=chunk)
                                nc.vector.match_replace(out=scr, in_to_replace=cand[:, c * DEPTH:c * DEPTH + 8],
                                                        in_values=chunk, imm_value=NEG)
                                nc.vector.max(out=cand[:, c * DEPTH + 8:c * DEPTH + 16], in_=scr)
                            topv = psel.tile([P, KSEL], f32, tag="topv")
                            idx_sel = psel.tile([P, KSEL], u16, tag="idx_sel")
                            cscr = psel.tile([P, NCH * DEPTH], f32, tag="cscr")
                            cur = cand
                            for r in range(KSEL // 8):
                                tv = topv[:, r * 8:(r + 1) * 8]
                                nc.vector.max(out=tv, in_=cur)
                                nc.vector.max_index(out=idx_sel[:, r * 8:(r + 1) * 8],
                                                    in_max=tv, in_values=scp)
                                if r < KSEL // 8 - 1:
                                    nxt = cscr if cur is cand else cand
                                    nc.vector.match_replace(out=nxt, in_to_replace=tv,
                                                            in_values=cur, imm_value=NEG)
                                    cur = nxt

                            # wrapped j-idx build (8 + 3 DMAs)
                            it = idx_tiles[t]
                            for q in range(8):
                                nc.sync.dma_start(it[0:16, KNN * q:KNN * (q + 1)],
                                                  idx_sel[16 * q:16 * (q + 1), 0:KNN])
                            for rep in range(1, 4):
                                nc.sync.dma_start(it[32 * rep:32 * rep + 16, :], it[0:16, :])

                        def stage_c(t):
                            it = idx_tiles[t]
                            # gather (dst limited to 1024 elem/partition per inst)
                            G = work.tile([P, P * KNN], f32, tag="G")
                            for g in range(4):
                                nc.gpsimd.indirect_copy(
                                    out=G[:, 960 * g:960 * (g + 1)], data=D,
                                    idxs=it[:, 60 * g:60 * (g + 1)],
                                    i_know_ap_gather_is_preferred=True)

                            # edge MLP + aggregate per sub-chunk
                            for q in range(NSUB):
                                gsl = G[:, q * ESUB:(q + 1) * ESUB]
                                h1p = edge_ps.tile([64, ESUB], f32, tag="h1p")
                                nc.tensor.matmul(h1p, lhsT=cw["wh"],
                                                 rhs=gsl[0:cw["kdim"], :],
                                                 start=True, stop=True)
                                h1 = work.tile([64, ESUB], f32, tag="h1")
                                nc.scalar.activation(h1, h1p, Act.Relu, bias=cw["b0"])
                                h2p = edge_ps.tile([64, ESUB], f32, tag="h2p")
                                nc.tensor.matmul(h2p, lhsT=cw["w1"], rhs=h1,
                                                 start=True, stop=True)
                                # max over the 30 edges of each point: layout [64, 30k, 16r]
                                h2v = h2p.rearrange("p (k r) -> p r k", r=16)
                                colsl = slice(t * P + q * SUB, t * P + (q + 1) * SUB)
                                red = work.tile([64, SUB], f32, tag="red")
                                nc.vector.tensor_reduce(out=red, in_=h2v,
                                                        axis=mybir.AxisListType.X,
                                                        op=Alu.max)
                                xdst = x3 if l == 2 else xt_next
                                nc.vector.tensor_scalar_add(xdst[0:64, colsl], red, cw["b1"])

                        for k in range(NT + 2):
                            if k < NT:
                                stage_a(k)
                            if 1 <= k <= NT:
                                stage_b(k - 1)
                            if k >= 2:
                                stage_c(k - 2)

                        # copy x_out into MLP input stack
                        if l == 0:
                            nc.sync.dma_start(x12[0:64, :], xt2[0:64, :])
                        elif l == 1:
                            nc.sync.dma_start(x12[64:128, :], xt3[0:64, :])

                # ---------------- pointwise MLP ----------------
                with ExitStack() as mctx:
                    mp = mctx.enter_context(tc.tile_pool(name=f"mlp{cloud}", bufs=2))
                    mps = mctx.enter_context(tc.tile_pool(name=f"mlpp{cloud}", bufs=4, space="PSUM"))
                    NCHK = 512
                    for nchk in range(N // NCHK):
                        csl = slice(nchk * NCHK, (nchk + 1) * NCHK)
                        h1m = mp.tile([128, 8, NCHK], f32, tag="h1m")
                        for m in range(8):
                            msl = slice(m * 128, (m + 1) * 128)
                            hp = mps.tile([128, NCHK], f32, tag="hp")
                            nc.tensor.matmul(hp, lhsT=mw0k0[:, msl], rhs=x12[:, csl],
                                             start=True, stop=False)
                            nc.tensor.matmul(hp, lhsT=mw0k1[:, msl], rhs=x3[:, csl],
                                             start=False, stop=True)
                            nc.scalar.activation(h1m[:, m, :], hp, Act.Relu, bias=mb0[:, m:m + 1])
                        h2m = mp.tile([128, 2, NCHK], f32, tag="h2m")
                        for m in range(2):
                            hp = mps.tile([128, NCHK], f32, tag="hp")
                            for s in range(8):
                                nc.tensor.matmul(hp, lhsT=mw1[:, s, m * 128:(m + 1) * 128],
                                                 rhs=h1m[:, s, :],
                                                 start=(s == 0), stop=(s == 7))
                            nc.scalar.activation(h2m[:, m, :], hp, Act.Relu, bias=mb1[:, m:m + 1])
                        hp3 = mps.tile([128, NCHK], f32, tag="hp")
                        for s in range(2):
                            nc.tensor.matmul(hp3, lhsT=mw2[:, s, :], rhs=h2m[:, s, :],
                                             start=(s == 0), stop=(s == 1))
                        h3m = mp.tile([128, NCHK], f32, tag="h3m")
                        nc.vector.tensor_scalar_add(h3m, hp3, mb2)
                        # final: out[pt, 3] = h3m[:, ptchunk].T @ finw + finb
                        for pchk in range(NCHK // 128):
                            fp = mps.tile([128, 3], f32, tag="fp")
                            nc.tensor.matmul(fp, lhsT=h3m[:, pchk * 128:(pchk + 1) * 128],
                                             rhs=finw, start=True, stop=True)
                            fo = mp.tile([128, 3], f32, tag="fo")
                            nc.vector.tensor_tensor(out=fo, in0=fp, in1=finb, op=Alu.add)
                            nc.sync.dma_start(
                                out_d[cloud, nchk * NCHK + pchk * 128:nchk * NCHK + (pchk + 1) * 128, :],
                                fo)
    nc.compile()
    return nc


def _host_inputs(inputs):
    """Build the per-core input maps (pure layout/indexing work)."""
    pos = np.ascontiguousarray(inputs["pos"], np.float32)

    iota_lc = np.broadcast_to(
        ((CHUNK - 1) - (np.arange(N) % CHUNK)).astype(np.uint32)[None, :], (P, N)).copy()

    # i-pattern wrapped index constant per row-tile:
    # edge slot s = 480q + 16k + r  ->  (pt = 16q + r, k);  wrapped[r, 30q + k] holds i(s)=t*128+16q+r
    iwrap = np.zeros((NT, 16, 8 * KNN), np.uint16)
    for t in range(NT):
        for r in range(16):
            for q in range(8):
                for k in range(KNN):
                    iwrap[t, r, KNN * q + k] = t * P + 16 * q + r

    def w(name):
        return np.ascontiguousarray(inputs[name], np.float32)

    common = {
        "iota_lc": iota_lc,
        "iwrap": iwrap,
        "mlp_w0k0": w("mlp_w0")[:128],
        "mlp_w0k1": w("mlp_w0")[128:],
        "mlp_b0": np.ascontiguousarray(w("mlp_b0").reshape(8, 128).T),
        "mlp_w1r": np.ascontiguousarray(w("mlp_w1").reshape(8, 128, 256).transpose(1, 0, 2)),
        "mlp_b1": np.ascontiguousarray(w("mlp_b1").reshape(2, 128).T),
        "mlp_w2r": np.ascontiguousarray(w("mlp_w2").reshape(2, 128, 128).transpose(1, 0, 2)),
        "mlp_b2": w("mlp_b2").reshape(128, 1),
        "fin_w": w("fin_w"),
        "fin_brep": np.broadcast_to(w("fin_b")[None, :], (128, 3)).copy(),
    }
    for l, c in ((1, 3), (2, 64), (3, 64)):
        w0 = w(f"c{l}_w0")
        common[f"c{l}_w0a"] = np.ascontiguousarray(w0[:c])
        common[f"c{l}_w0b"] = np.ascontiguousarray(w0[c:])
        common[f"c{l}_w1"] = w(f"c{l}_w1")
        common[f"c{l}_b0"] = w(f"c{l}_b0").reshape(64, 1)
        common[f"c{l}_b1"] = w(f"c{l}_b1").reshape(64, 1)

    in_maps = []
    for core in range(NCORES):
        m = dict(common)
        m["posT"] = np.ascontiguousarray(
            pos[core * CPC:(core + 1) * CPC].transpose(0, 2, 1))
        in_maps.append(m)
    return in_maps


def kernel(**inputs):
    from concourse.bass_utils import run_bass_kernel_spmd

    key = "prog"
    if key not in _PROGRAM_CACHE:
        _PROGRAM_CACHE[key] = _build_program()
    nc = _PROGRAM_CACHE[key]

    in_maps = _host_inputs(inputs)
    res = run_bass_kernel_spmd(nc, in_maps, list(range(NCORES)))
    outs = [res.results[i]["out"] for i in range(NCORES)]
    return np.concatenate(outs, axis=0).astype(np.float32)


if __name__ == "__main__":
    rng = np.random.default_rng(0)
    fake = {"pos": rng.standard_normal((B, N, 3), np.float32)}
    # quick build-only check
    _build_program()
    print("program built ok")

